# revision 10
# baseline (speedup 1.0000x reference)
"""Bass/Trainium2 kernel for nn_Attention_47622597378289.

Two chained attention blocks (encoder, decoder) over [B=8, C=512, H=W=48].
Data-parallel over batch: core i handles batch item i (B == n_cores == 8).

v3 design:
  - All matmul operands bf16 (host-cast): no fp32 HIGH/LOW_HIGH PE modes,
    FWL-fast weight loads throughout.
  - Out matmul computed TRANSPOSED: out[c,n] = sum_m vt[m,c].T @ exp[m,n].
    No PE transposes, x_enc lands directly in [c,n] layout for the decoder,
    and Out consumes exp chunks as they are produced (per-mi pipeline:
    ScalarE exp hides under PE work).
  - E matmuls row-packed 2x via tile_position (K=C8=64) with duplicated
    kp/q partition halves.
  - S via all-ones [128,128] lhsT -> arrives pre-broadcast [128,gw];
    1/S with DVE reciprocal_approx_fast; gamma folded into vt, gamma*bv
    folded as per-partition scalar in the residual STT.
  - Startup DMA round-robined over 3 HWDGE rings (~118GB/s each), wv
    first (V-proj is the first consumer), pos in bf16.
  - Decoder projections emitted one group LATE so the DVE/GpSimd residual
    ops for x_enc finish before PE needs them; residual STT on GpSimd.
"""

import numpy as np

import concourse.bass as bass
import concourse.bacc as bacc
import concourse.mybir as mybir
from concourse.bass_utils import run_bass_kernel_spmd
from concourse.tile import TileContext

F32 = mybir.dt.float32
BF16 = mybir.dt.bfloat16
AF = mybir.ActivationFunctionType
OP = mybir.AluOpType

B, C, H, W = 8, 512, 48, 48
C8 = C // 8          # 64
N = H * W            # 2304
P = 128
KC = C // P          # 4 c-chunks
NM = N // P          # 18 m-chunks
GROUPS = [(0, 512), (512, 512), (1024, 512), (1536, 512), (2048, 256)]
NQ = N // 4          # 576-col load quarters


def build_bass(gamma_e, gamma_d):
    nc = bacc.Bacc("TRN2", target_bir_lowering=False, debug=False)

    xbf_d = nc.dram_tensor("x_bf", [P, KC * N], BF16, kind="ExternalInput")
    tbf_d = nc.dram_tensor("tot_bf", [P, KC * N], BF16, kind="ExternalInput")
    xf_d = nc.dram_tensor("x_cn", [C, N], F32, kind="ExternalInput")
    wts_d = {}
    for p in ("e", "d"):
        wts_d[p] = {
            "wk": nc.dram_tensor(f"wk_{p}", [P, KC * P], BF16, kind="ExternalInput"),
            "wq": nc.dram_tensor(f"wq_{p}", [P, KC * P], BF16, kind="ExternalInput"),
            "wv": nc.dram_tensor(f"wv_{p}", [P, KC * C], BF16, kind="ExternalInput"),
            "pos": nc.dram_tensor(f"pos_{p}", [P, N], BF16, kind="ExternalInput"),
            "bq": nc.dram_tensor(f"bq_{p}", [P, 1], F32, kind="ExternalInput"),
            "gvb": nc.dram_tensor(f"gvb_{p}", [P, KC], F32, kind="ExternalInput"),
        }
    out_d = nc.dram_tensor("out_cn", [C, N], F32, kind="ExternalOutput")

    with TileContext(nc) as tc:
        import contextlib

        with contextlib.ExitStack() as ctx:
            pl = {
                "persist": ctx.enter_context(tc.tile_pool(name="persist", bufs=1)),
                "wpool": ctx.enter_context(tc.tile_pool(name="wpool", bufs=1)),
                "kq": ctx.enter_context(tc.tile_pool(name="kq", bufs=2)),
                "vt": ctx.enter_context(tc.tile_pool(name="vt", bufs=2)),
                "expe": ctx.enter_context(tc.tile_pool(name="expe", bufs=2)),
                "fbc": ctx.enter_context(tc.tile_pool(name="fbc", bufs=2)),
                "osb": ctx.enter_context(tc.tile_pool(name="osb", bufs=8)),
                "stream": ctx.enter_context(tc.tile_pool(name="stream", bufs=8)),
                "pp_e": ctx.enter_context(
                    tc.tile_pool(name="pp_e", bufs=3, space="PSUM")
                ),
                "pp_out": ctx.enter_context(
                    tc.tile_pool(name="pp_out", bufs=4, space="PSUM")
                ),
                "pp_s": ctx.enter_context(
                    tc.tile_pool(name="pp_s", bufs=1, space="PSUM")
                ),
            }
            wpool = pl["wpool"]
            persist = pl["persist"]

            ones = wpool.tile([P, P], BF16, tag="ones")
            nc.vector.memset(ones, 1.0)

            xs_bf = persist.tile([P, KC * N], BF16, tag="xs_bf")
            tot_bf = persist.tile([P, KC * N], BF16, tag="tot_bf")
            xenc_bf = persist.tile([P, KC * N], BF16, tag="xenc_bf")

            rings = [nc.sync, nc.scalar, nc.gpsimd]
            ring_i = [0]

            def dma_rr(out, in_):
                rings[ring_i[0] % 3].dma_start(out=out, in_=in_)
                ring_i[0] += 1

            def wtiles(p):
                return {
                    "wk": wpool.tile([P, KC * P], BF16, tag=f"wk{p}", name=f"wk_{p}"),
                    "wq": wpool.tile([P, KC * P], BF16, tag=f"wq{p}", name=f"wq_{p}"),
                    "wv": wpool.tile([P, KC * C], BF16, tag=f"wv{p}", name=f"wv_{p}"),
                    "pos": wpool.tile([P, N], BF16, tag=f"pos{p}", name=f"pos_{p}"),
                    "bq": wpool.tile([P, 1], F32, tag=f"bq{p}", name=f"bq_{p}"),
                    "gvb": wpool.tile([P, KC], F32, tag=f"gvb{p}", name=f"gvb_{p}"),
                }

            def load_wv(w, p):
                for c0, c1 in ((0, 768), (768, 1536), (1536, KC * C)):
                    dma_rr(w["wv"][:, c0:c1], wts_d[p]["wv"][:, c0:c1])

            def load_small(w, p):
                dma_rr(w["wk"], wts_d[p]["wk"][:, :])
                dma_rr(w["wq"], wts_d[p]["wq"][:, :])
                dma_rr(w["bq"], wts_d[p]["bq"][:, :])
                dma_rr(w["gvb"], wts_d[p]["gvb"][:, :])

            def load_pos(w, p):
                dma_rr(w["pos"][:, 0:1152], wts_d[p]["pos"][:, 0:1152])
                dma_rr(w["pos"][:, 1152:N], wts_d[p]["pos"][:, 1152:N])

            def load_big(dst, src):
                for q in range(4):
                    for k in range(KC):
                        sl = slice(k * N + q * NQ, k * N + (q + 1) * NQ)
                        dma_rr(dst[:, sl], src[:, sl])

            wt_e = wtiles("e")
            load_wv(wt_e, "e")           # first: V-proj is the first consumer
            load_big(xs_bf, xbf_d)
            load_small(wt_e, "e")
            load_pos(wt_e, "e")
            load_big(tot_bf, tbf_d)
            wt_d = wtiles("d")
            load_small(wt_d, "d")
            load_wv(wt_d, "d")
            load_pos(wt_d, "d")

            def proj_v(vt, wv, src, gamma, mi_range):
                for mi in mi_range:
                    vp = pl["pp_e"].tile([P, C], F32, tag="e", name=f"vp{mi}")
                    for k in range(KC):
                        nc.tensor.matmul(
                            vp,
                            src[:, k * N + mi * P : k * N + (mi + 1) * P],
                            wv[:, k * C : (k + 1) * C],
                            start=(k == 0),
                            stop=(k == KC - 1),
                        )
                    nc.vector.tensor_scalar_mul(
                        vt[:, mi * C : (mi + 1) * C], vp, float(gamma)
                    )

            def proj_k(kp, wk, pos, src, tiles):
                for n0, nw in tiles:
                    pp = pl["pp_e"].tile([P, 512], F32, tag="e", name="kpp")
                    for k in range(KC):
                        nc.tensor.matmul(
                            pp[:, :nw],
                            wk[:, k * P : (k + 1) * P],
                            src[:, k * N + n0 : k * N + n0 + nw],
                            start=(k == 0),
                            stop=(k == KC - 1),
                        )
                    nc.vector.tensor_add(
                        kp[:, n0 : n0 + nw], pp[:, :nw], pos[:, n0 : n0 + nw]
                    )

            def proj_q(qt, wq, bq, src):
                for n0, nw in GROUPS:
                    pp = pl["pp_e"].tile([P, 512], F32, tag="e", name="qpp")
                    for k in range(KC):
                        nc.tensor.matmul(
                            pp[:, :nw],
                            wq[:, k * P : (k + 1) * P],
                            src[:, k * N + n0 : k * N + n0 + nw],
                            start=(k == 0),
                            stop=(k == KC - 1),
                        )
                    nc.scalar.activation(
                        qt[:, n0 : n0 + nw], pp[:, :nw], AF.Identity, bias=bq
                    )

            def attn(kp, q, vt, wt, mode, post_group=None):
                """mode "enc": writes xenc_bf; "dec": DMAs out_cn."""
                enc = mode == "enc"
                NPAIR = NM // 2
                pairs = [(gi, pr) for gi in range(len(GROUPS))
                         for pr in range(0, NM, 2)]
                st = {}

                def group_state(gi):
                    if gi not in st:
                        n0, gw = GROUPS[gi]
                        exp_sb = pl["expe"].tile(
                            [P, NM * 512], BF16, tag="expe", name=f"exp_{mode}{gi}"
                        )
                        res_t = []
                        if not enc:
                            for kc in range(KC):
                                rt = pl["stream"].tile(
                                    [P, 512], F32, tag="res", name=f"res{kc}"
                                )
                                nc.gpsimd.dma_start(
                                    out=rt[:, :gw],
                                    in_=xf_d[kc * P : (kc + 1) * P, n0 : n0 + gw],
                                )
                                res_t.append(rt)
                        st[gi] = {"exp": exp_sb, "res": res_t, "s": None, "ops": None}
                    return st[gi]

                def epair(gi, pr):
                    n0, gw = GROUPS[gi]
                    g = group_state(gi)
                    ea = pl["pp_e"].tile([P, 512], F32, tag="e", name="ea")
                    eb = pl["pp_e"].tile([P, 512], F32, tag="e", name="eb")
                    nc.tensor.matmul(
                        ea[:, :gw],
                        kp[0:C8, pr * P : (pr + 1) * P],
                        q[0:C8, n0 : n0 + gw],
                        start=True,
                        stop=True,
                    )
                    nc.tensor.matmul(
                        eb[:, :gw],
                        kp[C8:P, (pr + 1) * P : (pr + 2) * P],
                        q[C8:P, n0 : n0 + gw],
                        start=True,
                        stop=True,
                    )
                    nc.scalar.activation(
                        g["exp"][:, pr * 512 : pr * 512 + gw], ea[:, :gw], AF.Exp
                    )
                    nc.scalar.activation(
                        g["exp"][:, (pr + 1) * 512 : (pr + 1) * 512 + gw],
                        eb[:, :gw],
                        AF.Exp,
                    )

                def boundary(gi):
                    n0, gw = GROUPS[gi]
                    g = st[gi]
                    fb = pl["fbc"].tile([P, 512], F32, tag="fbc", name="fbc")
                    nc.vector.reciprocal_approx_fast(fb[:, :gw], g["s"][:, :gw])
                    t1s = []
                    for kc in range(KC):
                        t1 = pl["osb"].tile([P, 512], F32, tag="osb", name=f"t1_{kc}")
                        nc.vector.tensor_mul(
                            t1[:, :gw], g["ops"][kc][:, :gw], fb[:, :gw]
                        )
                        t1s.append(t1)
                    if post_group is not None and gi >= 1:
                        post_group(gi - 1)
                    for kc in range(KC):
                        if enc:
                            nc.vector.scalar_tensor_tensor(
                                out=xenc_bf[:, kc * N + n0 : kc * N + n0 + gw],
                                in0=t1s[kc][:, :gw],
                                scalar=wt["gvb"][:, kc : kc + 1],
                                in1=xs_bf[:, kc * N + n0 : kc * N + n0 + gw],
                                op0=OP.add,
                                op1=OP.add,
                            )
                        else:
                            ro = pl["osb"].tile(
                                [P, 512], F32, tag="osb", name=f"ro_{kc}"
                            )
                            nc.vector.scalar_tensor_tensor(
                                out=ro[:, :gw],
                                in0=t1s[kc][:, :gw],
                                scalar=wt["gvb"][:, kc : kc + 1],
                                in1=g["res"][kc][:, :gw],
                                op0=OP.add,
                                op1=OP.add,
                            )
                            nc.sync.dma_start(
                                out=out_d[kc * P : (kc + 1) * P, n0 : n0 + gw],
                                in_=ro[:, :gw],
                            )

                epair(*pairs[0])
                epair(*pairs[1])
                for i, (gi, pr) in enumerate(pairs):
                    if i + 2 < len(pairs):
                        epair(*pairs[i + 2])
                    n0, gw = GROUPS[gi]
                    g = st[gi]
                    if g["s"] is None:
                        g["s"] = pl["pp_s"].tile([P, 512], F32, tag="s", name="s_ps")
                        g["ops"] = [
                            pl["pp_out"].tile([P, 512], F32, tag="out", name=f"o{kc}")
                            for kc in range(KC)
                        ]
                    exp_sb = g["exp"]
                    sl0 = pr * 512
                    sl1 = (pr + 1) * 512
                    nc.tensor.matmul(
                        g["s"][:, :gw],
                        ones,
                        exp_sb[:, sl0 : sl0 + gw],
                        start=(pr == 0),
                        stop=False,
                    )
                    nc.tensor.matmul(
                        g["s"][:, :gw],
                        ones,
                        exp_sb[:, sl1 : sl1 + gw],
                        start=False,
                        stop=(pr == NM - 2),
                    )
                    for kc in range(KC):
                        nc.tensor.matmul(
                            g["ops"][kc][:, :gw],
                            vt[:, (pr * KC + kc) * P : (pr * KC + kc + 1) * P],
                            exp_sb[:, sl0 : sl0 + gw],
                            start=(pr == 0),
                            stop=False,
                        )
                        nc.tensor.matmul(
                            g["ops"][kc][:, :gw],
                            vt[:, ((pr + 1) * KC + kc) * P : ((pr + 1) * KC + kc + 1) * P],
                            exp_sb[:, sl1 : sl1 + gw],
                            start=False,
                            stop=(pr == NM - 2),
                        )
                    if pr == NM - 2:
                        boundary(gi)
                if post_group is not None:
                    post_group(len(GROUPS) - 1)

            # ---- encoder projections ----
            vt_e = pl["vt"].tile([P, NM * C], BF16, tag="vt", name="vt_e")
            kp_e = pl["kq"].tile([P, N], BF16, tag="kp", name="kp_e")
            q_e = pl["kq"].tile([P, N], BF16, tag="q", name="q_e")
            proj_v(vt_e, wt_e["wv"], xs_bf, gamma_e, range(NM))
            proj_k(kp_e, wt_e["wk"], wt_e["pos"], xs_bf, GROUPS)

            vt_d = pl["vt"].tile([P, NM * C], BF16, tag="vt", name="vt_d")
            kp_d = pl["kq"].tile([P, N], BF16, tag="kp", name="kp_d")
            q_d = pl["kq"].tile([P, N], BF16, tag="q", name="q_d")
            proj_q(q_d, wt_d["wq"], wt_d["bq"], xs_bf)
            proj_q(q_e, wt_e["wq"], wt_e["bq"], tot_bf)

            def enc_post(gi):
                n0, gw = GROUPS[gi]
                proj_k(kp_d, wt_d["wk"], wt_d["pos"], xenc_bf, [(n0, gw)])
                proj_v(
                    vt_d, wt_d["wv"], xenc_bf, gamma_d,
                    range(n0 // P, (n0 + gw) // P),
                )

            attn(kp_e, q_e, vt_e, wt_e, "enc", post_group=enc_post)
            attn(kp_d, q_d, vt_d, wt_d, "dec")

    nc.compile()
    return nc


def kernel(**inputs):
    import ml_dtypes

    def to_bf(a):
        return np.ascontiguousarray(a).astype(ml_dtypes.bfloat16)

    x = np.asarray(inputs["x"], np.float32)
    total = np.asarray(inputs["total"], np.float32)

    def pack_cn(a_cn):
        out = np.empty((P, KC * a_cn.shape[1]), a_cn.dtype)
        M = a_cn.shape[1]
        for k in range(KC):
            out[:, k * M : (k + 1) * M] = a_cn[k * P : (k + 1) * P]
        return out

    def prep(pfx):
        Wq = np.asarray(inputs[f"{pfx}_Wq"], np.float32)
        bq = np.asarray(inputs[f"{pfx}_bq"], np.float32)
        Wk = np.asarray(inputs[f"{pfx}_Wk"], np.float32)
        bk = np.asarray(inputs[f"{pfx}_bk"], np.float32)
        Wv = np.asarray(inputs[f"{pfx}_Wv"], np.float32)
        bv = np.asarray(inputs[f"{pfx}_bv"], np.float32)
        ht = np.asarray(inputs[f"{pfx}_ht"], np.float32)
        wtt = np.asarray(inputs[f"{pfx}_wt"], np.float32)
        gamma = float(np.asarray(inputs[f"{pfx}_gamma"], np.float32).reshape(-1)[0])
        pos = (ht + wtt).reshape(C8, N) + bk[:, None]

        def dup_pack(wT):
            # [C, C8] -> [128, KC*128]: chunk k cols = [wT_k | wT_k]
            out = np.empty((P, KC * P), np.float32)
            for k in range(KC):
                blk = wT[k * P : (k + 1) * P]            # [128, 64]
                out[:, k * P : k * P + C8] = blk
                out[:, k * P + C8 : (k + 1) * P] = blk
            return out

        pos_dup = np.concatenate([pos, pos], axis=0)      # [128, N]
        bq_dup = np.concatenate([bq, bq]).reshape(P, 1)
        gvb = np.ascontiguousarray((gamma * bv).reshape(KC, P).T)  # [128, KC]
        return {
            "wk": to_bf(dup_pack(np.ascontiguousarray(Wk.T))),
            "wq": to_bf(dup_pack(np.ascontiguousarray(Wq.T))),
            "wv": to_bf(pack_cn(np.ascontiguousarray(Wv.T))),
            "pos": to_bf(pos_dup),
            "bq": np.ascontiguousarray(bq_dup),
            "gvb": gvb,
            "gamma": gamma,
        }

    pe, pd = prep("enc"), prep("dec")
    nc = build_bass(pe["gamma"], pd["gamma"])

    in_maps = []
    for b in range(B):
        x_cn = np.ascontiguousarray(x[b].reshape(C, N))
        tot_cn = np.ascontiguousarray(total[b].reshape(C, N))
        m = {
            "x_bf": to_bf(pack_cn(x_cn)),
            "tot_bf": to_bf(pack_cn(tot_cn)),
            "x_cn": x_cn,
        }
        for p, w in (("e", pe), ("d", pd)):
            for key in ("wk", "wq", "wv", "pos", "bq", "gvb"):
                m[f"{key}_{p}"] = w[key]
        in_maps.append(m)

    res = run_bass_kernel_spmd(nc, in_maps, core_ids=list(range(B)))
    out = np.stack(
        [res.results[b]["out_cn"].reshape(C, H, W) for b in range(B)], axis=0
    )
    return out.astype(np.float32)


if __name__ == "__main__":
    import reference

    ins = {k: np.asarray(v) for k, v in reference.setup_inputs().items()}
    got = kernel(**ins)
    exp = np.asarray(reference.reference(**ins))
    err = np.abs(got - exp).max() / (np.abs(exp).max() + 1e-30)
    print("abs-rel err:", err)


# revision 11
# speedup vs baseline: 1.0201x; 1.0201x over previous
"""Bass/Trainium2 kernel for nn_Attention_47622597378289.

Two chained attention blocks (encoder, decoder) over [B=8, C=512, H=W=48].
Data-parallel over batch: core i handles batch item i (B == n_cores == 8).

v3 design:
  - All matmul operands bf16 (host-cast): no fp32 HIGH/LOW_HIGH PE modes,
    FWL-fast weight loads throughout.
  - Out matmul computed TRANSPOSED: out[c,n] = sum_m vt[m,c].T @ exp[m,n].
    No PE transposes, x_enc lands directly in [c,n] layout for the decoder,
    and Out consumes exp chunks as they are produced (per-mi pipeline:
    ScalarE exp hides under PE work).
  - E matmuls row-packed 2x via tile_position (K=C8=64) with duplicated
    kp/q partition halves.
  - S via all-ones [128,128] lhsT -> arrives pre-broadcast [128,gw];
    1/S with DVE reciprocal_approx_fast; gamma folded into vt, gamma*bv
    folded as per-partition scalar in the residual STT.
  - Startup DMA round-robined over 3 HWDGE rings (~118GB/s each), wv
    first (V-proj is the first consumer), pos in bf16.
  - Decoder projections emitted one group LATE so the DVE/GpSimd residual
    ops for x_enc finish before PE needs them; residual STT on GpSimd.
"""

import numpy as np

import concourse.bass as bass
import concourse.bacc as bacc
import concourse.mybir as mybir
from concourse.bass_utils import run_bass_kernel_spmd
from concourse.tile import TileContext

F32 = mybir.dt.float32
BF16 = mybir.dt.bfloat16
AF = mybir.ActivationFunctionType
OP = mybir.AluOpType

B, C, H, W = 8, 512, 48, 48
C8 = C // 8          # 64
N = H * W            # 2304
P = 128
KC = C // P          # 4 c-chunks
NM = N // P          # 18 m-chunks
GROUPS = [(0, 512), (512, 512), (1024, 512), (1536, 512), (2048, 256)]
NQ = N // 4          # 576-col load quarters


def build_bass(gamma_e, gamma_d):
    nc = bacc.Bacc("TRN2", target_bir_lowering=False, debug=False)

    xbf_d = nc.dram_tensor("x_bf", [P, KC * N], BF16, kind="ExternalInput")
    tbf_d = nc.dram_tensor("tot_bf", [P, KC * N], BF16, kind="ExternalInput")
    xf_d = nc.dram_tensor("x_cn", [C, N], F32, kind="ExternalInput")
    wts_d = {}
    for p in ("e", "d"):
        wts_d[p] = {
            "wk": nc.dram_tensor(f"wk_{p}", [P, KC * P], BF16, kind="ExternalInput"),
            "wq": nc.dram_tensor(f"wq_{p}", [P, KC * P], BF16, kind="ExternalInput"),
            "wv": nc.dram_tensor(f"wv_{p}", [P, KC * C], BF16, kind="ExternalInput"),
            "pos": nc.dram_tensor(f"pos_{p}", [P, N], BF16, kind="ExternalInput"),
            "bq": nc.dram_tensor(f"bq_{p}", [P, 1], F32, kind="ExternalInput"),
            "gvb": nc.dram_tensor(f"gvb_{p}", [P, KC], F32, kind="ExternalInput"),
        }
    out_d = nc.dram_tensor("out_cn", [C, N], F32, kind="ExternalOutput")

    with TileContext(nc) as tc:
        import contextlib

        with contextlib.ExitStack() as ctx:
            pl = {
                "persist": ctx.enter_context(tc.tile_pool(name="persist", bufs=1)),
                "wpool": ctx.enter_context(tc.tile_pool(name="wpool", bufs=1)),
                "kq": ctx.enter_context(tc.tile_pool(name="kq", bufs=2)),
                "vt": ctx.enter_context(tc.tile_pool(name="vt", bufs=2)),
                "expe": ctx.enter_context(tc.tile_pool(name="expe", bufs=2)),
                "fbc": ctx.enter_context(tc.tile_pool(name="fbc", bufs=2)),
                "osb": ctx.enter_context(tc.tile_pool(name="osb", bufs=8)),
                "stream": ctx.enter_context(tc.tile_pool(name="stream", bufs=8)),
                "pp_e": ctx.enter_context(
                    tc.tile_pool(name="pp_e", bufs=3, space="PSUM")
                ),
                "pp_out": ctx.enter_context(
                    tc.tile_pool(name="pp_out", bufs=4, space="PSUM")
                ),
                "pp_s": ctx.enter_context(
                    tc.tile_pool(name="pp_s", bufs=1, space="PSUM")
                ),
            }
            wpool = pl["wpool"]
            persist = pl["persist"]

            ones = wpool.tile([P, P], BF16, tag="ones")
            nc.vector.memset(ones, 1.0)

            xs_bf = persist.tile([P, KC * N], BF16, tag="xs_bf")
            tot_bf = persist.tile([P, KC * N], BF16, tag="tot_bf")
            xenc_bf = persist.tile([P, KC * N], BF16, tag="xenc_bf")

            rings = [nc.sync, nc.scalar, nc.gpsimd]
            ring_i = [0]

            def dma_rr(out, in_):
                rings[ring_i[0] % 3].dma_start(out=out, in_=in_)
                ring_i[0] += 1

            def wtiles(p):
                return {
                    "wk": wpool.tile([P, KC * P], BF16, tag=f"wk{p}", name=f"wk_{p}"),
                    "wq": wpool.tile([P, KC * P], BF16, tag=f"wq{p}", name=f"wq_{p}"),
                    "wv": wpool.tile([P, KC * C], BF16, tag=f"wv{p}", name=f"wv_{p}"),
                    "pos": wpool.tile([P, N], BF16, tag=f"pos{p}", name=f"pos_{p}"),
                    "bq": wpool.tile([P, 1], F32, tag=f"bq{p}", name=f"bq_{p}"),
                    "gvb": wpool.tile([P, KC], F32, tag=f"gvb{p}", name=f"gvb_{p}"),
                }

            def load_wv(w, p):
                for c0, c1 in ((0, 768), (768, 1536), (1536, KC * C)):
                    dma_rr(w["wv"][:, c0:c1], wts_d[p]["wv"][:, c0:c1])

            def load_small(w, p):
                dma_rr(w["wk"], wts_d[p]["wk"][:, :])
                dma_rr(w["wq"], wts_d[p]["wq"][:, :])
                dma_rr(w["bq"], wts_d[p]["bq"][:, :])
                dma_rr(w["gvb"], wts_d[p]["gvb"][:, :])

            def load_pos(w, p):
                dma_rr(w["pos"][:, 0:1152], wts_d[p]["pos"][:, 0:1152])
                dma_rr(w["pos"][:, 1152:N], wts_d[p]["pos"][:, 1152:N])

            def load_big(dst, src):
                for q in range(4):
                    for k in range(KC):
                        sl = slice(k * N + q * NQ, k * N + (q + 1) * NQ)
                        dma_rr(dst[:, sl], src[:, sl])

            wt_e = wtiles("e")
            wt_d = wtiles("d")
            load_wv(wt_e, "e")           # first: V-proj is the first consumer
            load_big(xs_bf, xbf_d)
            load_small(wt_e, "e")
            load_small(wt_d, "d")        # before tot: decQ proj needs wq_d
            load_pos(wt_e, "e")
            load_big(tot_bf, tbf_d)
            load_wv(wt_d, "d")
            load_pos(wt_d, "d")

            def proj_v(vt, wv, src, gamma, mi_range):
                for mi in mi_range:
                    vp = pl["pp_e"].tile([P, C], F32, tag="e", name=f"vp{mi}")
                    for k in range(KC):
                        nc.tensor.matmul(
                            vp,
                            src[:, k * N + mi * P : k * N + (mi + 1) * P],
                            wv[:, k * C : (k + 1) * C],
                            start=(k == 0),
                            stop=(k == KC - 1),
                        )
                    nc.vector.tensor_scalar_mul(
                        vt[:, mi * C : (mi + 1) * C], vp, float(gamma)
                    )

            def proj_k(kp, wk, pos, src, tiles):
                for n0, nw in tiles:
                    pp = pl["pp_e"].tile([P, 512], F32, tag="e", name="kpp")
                    for k in range(KC):
                        nc.tensor.matmul(
                            pp[:, :nw],
                            wk[:, k * P : (k + 1) * P],
                            src[:, k * N + n0 : k * N + n0 + nw],
                            start=(k == 0),
                            stop=(k == KC - 1),
                        )
                    nc.vector.tensor_add(
                        kp[:, n0 : n0 + nw], pp[:, :nw], pos[:, n0 : n0 + nw]
                    )

            def proj_q(qt, wq, bq, src):
                for n0, nw in GROUPS:
                    pp = pl["pp_e"].tile([P, 512], F32, tag="e", name="qpp")
                    for k in range(KC):
                        nc.tensor.matmul(
                            pp[:, :nw],
                            wq[:, k * P : (k + 1) * P],
                            src[:, k * N + n0 : k * N + n0 + nw],
                            start=(k == 0),
                            stop=(k == KC - 1),
                        )
                    nc.scalar.activation(
                        qt[:, n0 : n0 + nw], pp[:, :nw], AF.Identity, bias=bq
                    )

            def attn(kp, q, vt, wt, mode, post_group=None):
                """mode "enc": writes xenc_bf; "dec": DMAs out_cn."""
                enc = mode == "enc"
                NPAIR = NM // 2
                pairs = [(gi, pr) for gi in range(len(GROUPS))
                         for pr in range(0, NM, 2)]
                st = {}

                def group_state(gi):
                    if gi not in st:
                        n0, gw = GROUPS[gi]
                        exp_sb = pl["expe"].tile(
                            [P, NM * 512], BF16, tag="expe", name=f"exp_{mode}{gi}"
                        )
                        res_t = []
                        if not enc:
                            for kc in range(KC):
                                rt = pl["stream"].tile(
                                    [P, 512], F32, tag="res", name=f"res{kc}"
                                )
                                nc.gpsimd.dma_start(
                                    out=rt[:, :gw],
                                    in_=xf_d[kc * P : (kc + 1) * P, n0 : n0 + gw],
                                )
                                res_t.append(rt)
                        st[gi] = {"exp": exp_sb, "res": res_t, "s": None, "ops": None}
                    return st[gi]

                def epair(gi, pr):
                    n0, gw = GROUPS[gi]
                    g = group_state(gi)
                    ea = pl["pp_e"].tile([P, 512], F32, tag="e", name="ea")
                    eb = pl["pp_e"].tile([P, 512], F32, tag="e", name="eb")
                    nc.tensor.matmul(
                        ea[:, :gw],
                        kp[0:C8, pr * P : (pr + 1) * P],
                        q[0:C8, n0 : n0 + gw],
                        start=True,
                        stop=True,
                    )
                    nc.tensor.matmul(
                        eb[:, :gw],
                        kp[C8:P, (pr + 1) * P : (pr + 2) * P],
                        q[C8:P, n0 : n0 + gw],
                        start=True,
                        stop=True,
                    )
                    nc.scalar.activation(
                        g["exp"][:, pr * 512 : pr * 512 + gw], ea[:, :gw], AF.Exp
                    )
                    nc.scalar.activation(
                        g["exp"][:, (pr + 1) * 512 : (pr + 1) * 512 + gw],
                        eb[:, :gw],
                        AF.Exp,
                    )

                def boundary(gi):
                    n0, gw = GROUPS[gi]
                    g = st[gi]
                    fb = pl["fbc"].tile([P, 512], F32, tag="fbc", name="fbc")
                    nc.vector.reciprocal_approx_fast(fb[:, :gw], g["s"][:, :gw])
                    t1s = []
                    for kc in range(KC):
                        t1 = pl["osb"].tile([P, 512], F32, tag="osb", name=f"t1_{kc}")
                        nc.vector.tensor_mul(
                            t1[:, :gw], g["ops"][kc][:, :gw], fb[:, :gw]
                        )
                        t1s.append(t1)
                    if post_group is not None and gi >= 1:
                        post_group(gi - 1)
                    for kc in range(KC):
                        if enc:
                            nc.vector.scalar_tensor_tensor(
                                out=xenc_bf[:, kc * N + n0 : kc * N + n0 + gw],
                                in0=t1s[kc][:, :gw],
                                scalar=wt["gvb"][:, kc : kc + 1],
                                in1=xs_bf[:, kc * N + n0 : kc * N + n0 + gw],
                                op0=OP.add,
                                op1=OP.add,
                            )
                        else:
                            ro = pl["osb"].tile(
                                [P, 512], F32, tag="osb", name=f"ro_{kc}"
                            )
                            nc.vector.scalar_tensor_tensor(
                                out=ro[:, :gw],
                                in0=t1s[kc][:, :gw],
                                scalar=wt["gvb"][:, kc : kc + 1],
                                in1=g["res"][kc][:, :gw],
                                op0=OP.add,
                                op1=OP.add,
                            )
                            nc.sync.dma_start(
                                out=out_d[kc * P : (kc + 1) * P, n0 : n0 + gw],
                                in_=ro[:, :gw],
                            )

                epair(*pairs[0])
                epair(*pairs[1])
                for i, (gi, pr) in enumerate(pairs):
                    if i + 2 < len(pairs):
                        epair(*pairs[i + 2])
                    n0, gw = GROUPS[gi]
                    g = st[gi]
                    if g["s"] is None:
                        g["s"] = pl["pp_s"].tile([P, 512], F32, tag="s", name="s_ps")
                        g["ops"] = [
                            pl["pp_out"].tile([P, 512], F32, tag="out", name=f"o{kc}")
                            for kc in range(KC)
                        ]
                    exp_sb = g["exp"]
                    sl0 = pr * 512
                    sl1 = (pr + 1) * 512
                    nc.tensor.matmul(
                        g["s"][:, :gw],
                        ones,
                        exp_sb[:, sl0 : sl0 + gw],
                        start=(pr == 0),
                        stop=False,
                    )
                    nc.tensor.matmul(
                        g["s"][:, :gw],
                        ones,
                        exp_sb[:, sl1 : sl1 + gw],
                        start=False,
                        stop=(pr == NM - 2),
                    )
                    for kc in range(KC):
                        nc.tensor.matmul(
                            g["ops"][kc][:, :gw],
                            vt[:, (pr * KC + kc) * P : (pr * KC + kc + 1) * P],
                            exp_sb[:, sl0 : sl0 + gw],
                            start=(pr == 0),
                            stop=False,
                        )
                        nc.tensor.matmul(
                            g["ops"][kc][:, :gw],
                            vt[:, ((pr + 1) * KC + kc) * P : ((pr + 1) * KC + kc + 1) * P],
                            exp_sb[:, sl1 : sl1 + gw],
                            start=False,
                            stop=(pr == NM - 2),
                        )
                    if pr == NM - 2:
                        boundary(gi)
                if post_group is not None:
                    post_group(len(GROUPS) - 1)

            # ---- encoder projections ----
            vt_e = pl["vt"].tile([P, NM * C], BF16, tag="vt", name="vt_e")
            kp_e = pl["kq"].tile([P, N], BF16, tag="kp", name="kp_e")
            q_e = pl["kq"].tile([P, N], BF16, tag="q", name="q_e")
            proj_v(vt_e, wt_e["wv"], xs_bf, gamma_e, range(NM))
            proj_k(kp_e, wt_e["wk"], wt_e["pos"], xs_bf, GROUPS)

            vt_d = pl["vt"].tile([P, NM * C], BF16, tag="vt", name="vt_d")
            kp_d = pl["kq"].tile([P, N], BF16, tag="kp", name="kp_d")
            q_d = pl["kq"].tile([P, N], BF16, tag="q", name="q_d")
            proj_q(q_d, wt_d["wq"], wt_d["bq"], xs_bf)
            proj_q(q_e, wt_e["wq"], wt_e["bq"], tot_bf)

            def enc_post(gi):
                n0, gw = GROUPS[gi]
                proj_k(kp_d, wt_d["wk"], wt_d["pos"], xenc_bf, [(n0, gw)])
                proj_v(
                    vt_d, wt_d["wv"], xenc_bf, gamma_d,
                    range(n0 // P, (n0 + gw) // P),
                )

            attn(kp_e, q_e, vt_e, wt_e, "enc", post_group=enc_post)
            attn(kp_d, q_d, vt_d, wt_d, "dec")

    nc.compile()
    return nc


def kernel(**inputs):
    import ml_dtypes

    def to_bf(a):
        return np.ascontiguousarray(a).astype(ml_dtypes.bfloat16)

    x = np.asarray(inputs["x"], np.float32)
    total = np.asarray(inputs["total"], np.float32)

    def pack_cn(a_cn):
        out = np.empty((P, KC * a_cn.shape[1]), a_cn.dtype)
        M = a_cn.shape[1]
        for k in range(KC):
            out[:, k * M : (k + 1) * M] = a_cn[k * P : (k + 1) * P]
        return out

    def prep(pfx):
        Wq = np.asarray(inputs[f"{pfx}_Wq"], np.float32)
        bq = np.asarray(inputs[f"{pfx}_bq"], np.float32)
        Wk = np.asarray(inputs[f"{pfx}_Wk"], np.float32)
        bk = np.asarray(inputs[f"{pfx}_bk"], np.float32)
        Wv = np.asarray(inputs[f"{pfx}_Wv"], np.float32)
        bv = np.asarray(inputs[f"{pfx}_bv"], np.float32)
        ht = np.asarray(inputs[f"{pfx}_ht"], np.float32)
        wtt = np.asarray(inputs[f"{pfx}_wt"], np.float32)
        gamma = float(np.asarray(inputs[f"{pfx}_gamma"], np.float32).reshape(-1)[0])
        pos = (ht + wtt).reshape(C8, N) + bk[:, None]

        def dup_pack(wT):
            # [C, C8] -> [128, KC*128]: chunk k cols = [wT_k | wT_k]
            out = np.empty((P, KC * P), np.float32)
            for k in range(KC):
                blk = wT[k * P : (k + 1) * P]            # [128, 64]
                out[:, k * P : k * P + C8] = blk
                out[:, k * P + C8 : (k + 1) * P] = blk
            return out

        pos_dup = np.concatenate([pos, pos], axis=0)      # [128, N]
        bq_dup = np.concatenate([bq, bq]).reshape(P, 1)
        gvb = np.ascontiguousarray((gamma * bv).reshape(KC, P).T)  # [128, KC]
        return {
            "wk": to_bf(dup_pack(np.ascontiguousarray(Wk.T))),
            "wq": to_bf(dup_pack(np.ascontiguousarray(Wq.T))),
            "wv": to_bf(pack_cn(np.ascontiguousarray(Wv.T))),
            "pos": to_bf(pos_dup),
            "bq": np.ascontiguousarray(bq_dup),
            "gvb": gvb,
            "gamma": gamma,
        }

    pe, pd = prep("enc"), prep("dec")
    nc = build_bass(pe["gamma"], pd["gamma"])

    in_maps = []
    for b in range(B):
        x_cn = np.ascontiguousarray(x[b].reshape(C, N))
        tot_cn = np.ascontiguousarray(total[b].reshape(C, N))
        m = {
            "x_bf": to_bf(pack_cn(x_cn)),
            "tot_bf": to_bf(pack_cn(tot_cn)),
            "x_cn": x_cn,
        }
        for p, w in (("e", pe), ("d", pd)):
            for key in ("wk", "wq", "wv", "pos", "bq", "gvb"):
                m[f"{key}_{p}"] = w[key]
        in_maps.append(m)

    res = run_bass_kernel_spmd(nc, in_maps, core_ids=list(range(B)))
    out = np.stack(
        [res.results[b]["out_cn"].reshape(C, H, W) for b in range(B)], axis=0
    )
    return out.astype(np.float32)


if __name__ == "__main__":
    import reference

    ins = {k: np.asarray(v) for k, v in reference.setup_inputs().items()}
    got = kernel(**ins)
    exp = np.asarray(reference.reference(**ins))
    err = np.abs(got - exp).max() / (np.abs(exp).max() + 1e-30)
    print("abs-rel err:", err)


# revision 12
# speedup vs baseline: 1.0225x; 1.0024x over previous
"""Bass/Trainium2 kernel for nn_Attention_47622597378289.

Two chained attention blocks (encoder, decoder) over [B=8, C=512, H=W=48].
Data-parallel over batch: core i handles batch item i (B == n_cores == 8).

v3 design:
  - All matmul operands bf16 (host-cast): no fp32 HIGH/LOW_HIGH PE modes,
    FWL-fast weight loads throughout.
  - Out matmul computed TRANSPOSED: out[c,n] = sum_m vt[m,c].T @ exp[m,n].
    No PE transposes, x_enc lands directly in [c,n] layout for the decoder,
    and Out consumes exp chunks as they are produced (per-mi pipeline:
    ScalarE exp hides under PE work).
  - E matmuls row-packed 2x via tile_position (K=C8=64) with duplicated
    kp/q partition halves.
  - S via all-ones [128,128] lhsT -> arrives pre-broadcast [128,gw];
    1/S with DVE reciprocal_approx_fast; gamma folded into vt, gamma*bv
    folded as per-partition scalar in the residual STT.
  - Startup DMA round-robined over 3 HWDGE rings (~118GB/s each), wv
    first (V-proj is the first consumer), pos in bf16.
  - Decoder projections emitted one group LATE so the DVE/GpSimd residual
    ops for x_enc finish before PE needs them; residual STT on GpSimd.
"""

import numpy as np

import concourse.bass as bass
import concourse.bacc as bacc
import concourse.mybir as mybir
from concourse.bass_utils import run_bass_kernel_spmd
from concourse.tile import TileContext

F32 = mybir.dt.float32
BF16 = mybir.dt.bfloat16
AF = mybir.ActivationFunctionType
OP = mybir.AluOpType

B, C, H, W = 8, 512, 48, 48
C8 = C // 8          # 64
N = H * W            # 2304
P = 128
KC = C // P          # 4 c-chunks
NM = N // P          # 18 m-chunks
GROUPS = [(0, 512), (512, 512), (1024, 512), (1536, 512), (2048, 256)]
NQ = N // 4          # 576-col load quarters


def build_bass(gamma_e, gamma_d):
    nc = bacc.Bacc("TRN2", target_bir_lowering=False, debug=False)

    xbf_d = nc.dram_tensor("x_bf", [P, KC * N], BF16, kind="ExternalInput")
    tbf_d = nc.dram_tensor("tot_bf", [P, KC * N], BF16, kind="ExternalInput")
    xf_d = nc.dram_tensor("x_cn", [C, N], F32, kind="ExternalInput")
    wts_d = {}
    for p in ("e", "d"):
        wts_d[p] = {
            "wk": nc.dram_tensor(f"wk_{p}", [P, KC * P], BF16, kind="ExternalInput"),
            "wq": nc.dram_tensor(f"wq_{p}", [P, KC * P], BF16, kind="ExternalInput"),
            "wv": nc.dram_tensor(f"wv_{p}", [P, KC * C], BF16, kind="ExternalInput"),
            "pos": nc.dram_tensor(f"pos_{p}", [P, N], BF16, kind="ExternalInput"),
            "bq": nc.dram_tensor(f"bq_{p}", [P, 1], F32, kind="ExternalInput"),
            "gvb": nc.dram_tensor(f"gvb_{p}", [P, KC], F32, kind="ExternalInput"),
        }
    out_d = nc.dram_tensor("out_cn", [C, N], F32, kind="ExternalOutput")

    with TileContext(nc) as tc:
        import contextlib

        with contextlib.ExitStack() as ctx:
            pl = {
                "persist": ctx.enter_context(tc.tile_pool(name="persist", bufs=1)),
                "wpool": ctx.enter_context(tc.tile_pool(name="wpool", bufs=1)),
                "kq": ctx.enter_context(tc.tile_pool(name="kq", bufs=2)),
                "vt": ctx.enter_context(tc.tile_pool(name="vt", bufs=2)),
                "expe": ctx.enter_context(tc.tile_pool(name="expe", bufs=2)),
                "fbc": ctx.enter_context(tc.tile_pool(name="fbc", bufs=2)),
                "osb": ctx.enter_context(tc.tile_pool(name="osb", bufs=8)),
                "stream": ctx.enter_context(tc.tile_pool(name="stream", bufs=8)),
                "pp_e": ctx.enter_context(
                    tc.tile_pool(name="pp_e", bufs=3, space="PSUM")
                ),
                "pp_out": ctx.enter_context(
                    tc.tile_pool(name="pp_out", bufs=4, space="PSUM")
                ),
                "pp_s": ctx.enter_context(
                    tc.tile_pool(name="pp_s", bufs=1, space="PSUM")
                ),
            }
            wpool = pl["wpool"]
            persist = pl["persist"]

            ones = wpool.tile([P, P], BF16, tag="ones")
            nc.vector.memset(ones, 1.0)

            xs_bf = persist.tile([P, KC * N], BF16, tag="xs_bf")
            tot_bf = persist.tile([P, KC * N], BF16, tag="tot_bf")
            xenc_bf = persist.tile([P, KC * N], BF16, tag="xenc_bf")

            rings = [nc.sync, nc.scalar, nc.gpsimd]
            ring_i = [0]

            def dma_rr(out, in_):
                rings[ring_i[0] % 3].dma_start(out=out, in_=in_)
                ring_i[0] += 1

            def wtiles(p):
                return {
                    "wk": wpool.tile([P, KC * P], BF16, tag=f"wk{p}", name=f"wk_{p}"),
                    "wq": wpool.tile([P, KC * P], BF16, tag=f"wq{p}", name=f"wq_{p}"),
                    "wv": wpool.tile([P, KC * C], BF16, tag=f"wv{p}", name=f"wv_{p}"),
                    "pos": wpool.tile([P, N], BF16, tag=f"pos{p}", name=f"pos_{p}"),
                    "bq": wpool.tile([P, 1], F32, tag=f"bq{p}", name=f"bq_{p}"),
                    "gvb": wpool.tile([P, KC], F32, tag=f"gvb{p}", name=f"gvb_{p}"),
                }

            def load_wv(w, p):
                for c0, c1 in ((0, 768), (768, 1536), (1536, KC * C)):
                    dma_rr(w["wv"][:, c0:c1], wts_d[p]["wv"][:, c0:c1])

            def load_small(w, p):
                dma_rr(w["wk"], wts_d[p]["wk"][:, :])
                dma_rr(w["wq"], wts_d[p]["wq"][:, :])
                dma_rr(w["bq"], wts_d[p]["bq"][:, :])
                dma_rr(w["gvb"], wts_d[p]["gvb"][:, :])

            def load_pos(w, p):
                dma_rr(w["pos"][:, 0:1152], wts_d[p]["pos"][:, 0:1152])
                dma_rr(w["pos"][:, 1152:N], wts_d[p]["pos"][:, 1152:N])

            def load_big(dst, src, fine_head=False):
                bounds = ([0, 288, NQ] if fine_head else [0, NQ]) + [
                    2 * NQ, 3 * NQ, N
                ]
                for c0, c1 in zip(bounds[:-1], bounds[1:]):
                    for k in range(KC):
                        sl = slice(k * N + c0, k * N + c1)
                        dma_rr(dst[:, sl], src[:, sl])

            wt_e = wtiles("e")
            wt_d = wtiles("d")
            load_wv(wt_e, "e")           # first: V-proj is the first consumer
            load_big(xs_bf, xbf_d, fine_head=True)
            load_small(wt_e, "e")
            load_small(wt_d, "d")        # before tot: decQ proj needs wq_d
            load_pos(wt_e, "e")
            load_big(tot_bf, tbf_d)
            load_wv(wt_d, "d")
            load_pos(wt_d, "d")

            def proj_v(vt, wv, src, gamma, mi_range):
                for mi in mi_range:
                    vp = pl["pp_e"].tile([P, C], F32, tag="e", name=f"vp{mi}")
                    for k in range(KC):
                        nc.tensor.matmul(
                            vp,
                            src[:, k * N + mi * P : k * N + (mi + 1) * P],
                            wv[:, k * C : (k + 1) * C],
                            start=(k == 0),
                            stop=(k == KC - 1),
                        )
                    nc.vector.tensor_scalar_mul(
                        vt[:, mi * C : (mi + 1) * C], vp, float(gamma)
                    )

            def proj_k(kp, wk, pos, src, tiles):
                for n0, nw in tiles:
                    pp = pl["pp_e"].tile([P, 512], F32, tag="e", name="kpp")
                    for k in range(KC):
                        nc.tensor.matmul(
                            pp[:, :nw],
                            wk[:, k * P : (k + 1) * P],
                            src[:, k * N + n0 : k * N + n0 + nw],
                            start=(k == 0),
                            stop=(k == KC - 1),
                        )
                    nc.vector.tensor_add(
                        kp[:, n0 : n0 + nw], pp[:, :nw], pos[:, n0 : n0 + nw]
                    )

            def proj_q(qt, wq, bq, src):
                for n0, nw in GROUPS:
                    pp = pl["pp_e"].tile([P, 512], F32, tag="e", name="qpp")
                    for k in range(KC):
                        nc.tensor.matmul(
                            pp[:, :nw],
                            wq[:, k * P : (k + 1) * P],
                            src[:, k * N + n0 : k * N + n0 + nw],
                            start=(k == 0),
                            stop=(k == KC - 1),
                        )
                    nc.scalar.activation(
                        qt[:, n0 : n0 + nw], pp[:, :nw], AF.Identity, bias=bq
                    )

            def attn(kp, q, vt, wt, mode, post_group=None):
                """mode "enc": writes xenc_bf; "dec": DMAs out_cn."""
                enc = mode == "enc"
                NPAIR = NM // 2
                pairs = [(gi, pr) for gi in range(len(GROUPS))
                         for pr in range(0, NM, 2)]
                st = {}

                def group_state(gi):
                    if gi not in st:
                        n0, gw = GROUPS[gi]
                        exp_sb = pl["expe"].tile(
                            [P, NM * 512], BF16, tag="expe", name=f"exp_{mode}{gi}"
                        )
                        res_t = []
                        if not enc:
                            for kc in range(KC):
                                rt = pl["stream"].tile(
                                    [P, 512], F32, tag="res", name=f"res{kc}"
                                )
                                nc.gpsimd.dma_start(
                                    out=rt[:, :gw],
                                    in_=xf_d[kc * P : (kc + 1) * P, n0 : n0 + gw],
                                )
                                res_t.append(rt)
                        st[gi] = {"exp": exp_sb, "res": res_t, "s": None, "ops": None}
                    return st[gi]

                def epair(gi, pr):
                    n0, gw = GROUPS[gi]
                    g = group_state(gi)
                    ea = pl["pp_e"].tile([P, 512], F32, tag="e", name="ea")
                    eb = pl["pp_e"].tile([P, 512], F32, tag="e", name="eb")
                    nc.tensor.matmul(
                        ea[:, :gw],
                        kp[0:C8, pr * P : (pr + 1) * P],
                        q[0:C8, n0 : n0 + gw],
                        start=True,
                        stop=True,
                    )
                    nc.tensor.matmul(
                        eb[:, :gw],
                        kp[C8:P, (pr + 1) * P : (pr + 2) * P],
                        q[C8:P, n0 : n0 + gw],
                        start=True,
                        stop=True,
                    )
                    nc.scalar.activation(
                        g["exp"][:, pr * 512 : pr * 512 + gw], ea[:, :gw], AF.Exp
                    )
                    nc.scalar.activation(
                        g["exp"][:, (pr + 1) * 512 : (pr + 1) * 512 + gw],
                        eb[:, :gw],
                        AF.Exp,
                    )

                def boundary(gi):
                    n0, gw = GROUPS[gi]
                    g = st[gi]
                    fb = pl["fbc"].tile([P, 512], F32, tag="fbc", name="fbc")
                    nc.vector.reciprocal_approx_fast(fb[:, :gw], g["s"][:, :gw])
                    t1s = []
                    for kc in range(KC):
                        t1 = pl["osb"].tile([P, 512], F32, tag="osb", name=f"t1_{kc}")
                        nc.vector.tensor_mul(
                            t1[:, :gw], g["ops"][kc][:, :gw], fb[:, :gw]
                        )
                        t1s.append(t1)
                    if post_group is not None and gi >= 1:
                        post_group(gi - 1)
                    for kc in range(KC):
                        if enc:
                            nc.vector.scalar_tensor_tensor(
                                out=xenc_bf[:, kc * N + n0 : kc * N + n0 + gw],
                                in0=t1s[kc][:, :gw],
                                scalar=wt["gvb"][:, kc : kc + 1],
                                in1=xs_bf[:, kc * N + n0 : kc * N + n0 + gw],
                                op0=OP.add,
                                op1=OP.add,
                            )
                        else:
                            ro = pl["osb"].tile(
                                [P, 512], F32, tag="osb", name=f"ro_{kc}"
                            )
                            nc.vector.scalar_tensor_tensor(
                                out=ro[:, :gw],
                                in0=t1s[kc][:, :gw],
                                scalar=wt["gvb"][:, kc : kc + 1],
                                in1=g["res"][kc][:, :gw],
                                op0=OP.add,
                                op1=OP.add,
                            )
                            (nc.sync if kc % 2 == 0 else nc.gpsimd).dma_start(
                                out=out_d[kc * P : (kc + 1) * P, n0 : n0 + gw],
                                in_=ro[:, :gw],
                            )

                epair(*pairs[0])
                epair(*pairs[1])
                for i, (gi, pr) in enumerate(pairs):
                    if i + 2 < len(pairs):
                        epair(*pairs[i + 2])
                    n0, gw = GROUPS[gi]
                    g = st[gi]
                    if g["s"] is None:
                        g["s"] = pl["pp_s"].tile([P, 512], F32, tag="s", name="s_ps")
                        g["ops"] = [
                            pl["pp_out"].tile([P, 512], F32, tag="out", name=f"o{kc}")
                            for kc in range(KC)
                        ]
                    exp_sb = g["exp"]
                    sl0 = pr * 512
                    sl1 = (pr + 1) * 512
                    nc.tensor.matmul(
                        g["s"][:, :gw],
                        ones,
                        exp_sb[:, sl0 : sl0 + gw],
                        start=(pr == 0),
                        stop=False,
                    )
                    nc.tensor.matmul(
                        g["s"][:, :gw],
                        ones,
                        exp_sb[:, sl1 : sl1 + gw],
                        start=False,
                        stop=(pr == NM - 2),
                    )
                    for kc in range(KC):
                        nc.tensor.matmul(
                            g["ops"][kc][:, :gw],
                            vt[:, (pr * KC + kc) * P : (pr * KC + kc + 1) * P],
                            exp_sb[:, sl0 : sl0 + gw],
                            start=(pr == 0),
                            stop=False,
                        )
                        nc.tensor.matmul(
                            g["ops"][kc][:, :gw],
                            vt[:, ((pr + 1) * KC + kc) * P : ((pr + 1) * KC + kc + 1) * P],
                            exp_sb[:, sl1 : sl1 + gw],
                            start=False,
                            stop=(pr == NM - 2),
                        )
                    if pr == NM - 2:
                        boundary(gi)
                if post_group is not None:
                    post_group(len(GROUPS) - 1)

            # ---- encoder projections ----
            vt_e = pl["vt"].tile([P, NM * C], BF16, tag="vt", name="vt_e")
            kp_e = pl["kq"].tile([P, N], BF16, tag="kp", name="kp_e")
            q_e = pl["kq"].tile([P, N], BF16, tag="q", name="q_e")
            proj_v(vt_e, wt_e["wv"], xs_bf, gamma_e, range(NM))
            proj_k(kp_e, wt_e["wk"], wt_e["pos"], xs_bf, GROUPS)

            vt_d = pl["vt"].tile([P, NM * C], BF16, tag="vt", name="vt_d")
            kp_d = pl["kq"].tile([P, N], BF16, tag="kp", name="kp_d")
            q_d = pl["kq"].tile([P, N], BF16, tag="q", name="q_d")
            proj_q(q_d, wt_d["wq"], wt_d["bq"], xs_bf)
            proj_q(q_e, wt_e["wq"], wt_e["bq"], tot_bf)

            def enc_post(gi):
                n0, gw = GROUPS[gi]
                proj_k(kp_d, wt_d["wk"], wt_d["pos"], xenc_bf, [(n0, gw)])
                proj_v(
                    vt_d, wt_d["wv"], xenc_bf, gamma_d,
                    range(n0 // P, (n0 + gw) // P),
                )

            attn(kp_e, q_e, vt_e, wt_e, "enc", post_group=enc_post)
            attn(kp_d, q_d, vt_d, wt_d, "dec")

    nc.compile()
    return nc


def kernel(**inputs):
    import ml_dtypes

    def to_bf(a):
        return np.ascontiguousarray(a).astype(ml_dtypes.bfloat16)

    x = np.asarray(inputs["x"], np.float32)
    total = np.asarray(inputs["total"], np.float32)

    def pack_cn(a_cn):
        out = np.empty((P, KC * a_cn.shape[1]), a_cn.dtype)
        M = a_cn.shape[1]
        for k in range(KC):
            out[:, k * M : (k + 1) * M] = a_cn[k * P : (k + 1) * P]
        return out

    def prep(pfx):
        Wq = np.asarray(inputs[f"{pfx}_Wq"], np.float32)
        bq = np.asarray(inputs[f"{pfx}_bq"], np.float32)
        Wk = np.asarray(inputs[f"{pfx}_Wk"], np.float32)
        bk = np.asarray(inputs[f"{pfx}_bk"], np.float32)
        Wv = np.asarray(inputs[f"{pfx}_Wv"], np.float32)
        bv = np.asarray(inputs[f"{pfx}_bv"], np.float32)
        ht = np.asarray(inputs[f"{pfx}_ht"], np.float32)
        wtt = np.asarray(inputs[f"{pfx}_wt"], np.float32)
        gamma = float(np.asarray(inputs[f"{pfx}_gamma"], np.float32).reshape(-1)[0])
        pos = (ht + wtt).reshape(C8, N) + bk[:, None]

        def dup_pack(wT):
            # [C, C8] -> [128, KC*128]: chunk k cols = [wT_k | wT_k]
            out = np.empty((P, KC * P), np.float32)
            for k in range(KC):
                blk = wT[k * P : (k + 1) * P]            # [128, 64]
                out[:, k * P : k * P + C8] = blk
                out[:, k * P + C8 : (k + 1) * P] = blk
            return out

        pos_dup = np.concatenate([pos, pos], axis=0)      # [128, N]
        bq_dup = np.concatenate([bq, bq]).reshape(P, 1)
        gvb = np.ascontiguousarray((gamma * bv).reshape(KC, P).T)  # [128, KC]
        return {
            "wk": to_bf(dup_pack(np.ascontiguousarray(Wk.T))),
            "wq": to_bf(dup_pack(np.ascontiguousarray(Wq.T))),
            "wv": to_bf(pack_cn(np.ascontiguousarray(Wv.T))),
            "pos": to_bf(pos_dup),
            "bq": np.ascontiguousarray(bq_dup),
            "gvb": gvb,
            "gamma": gamma,
        }

    pe, pd = prep("enc"), prep("dec")
    nc = build_bass(pe["gamma"], pd["gamma"])

    in_maps = []
    for b in range(B):
        x_cn = np.ascontiguousarray(x[b].reshape(C, N))
        tot_cn = np.ascontiguousarray(total[b].reshape(C, N))
        m = {
            "x_bf": to_bf(pack_cn(x_cn)),
            "tot_bf": to_bf(pack_cn(tot_cn)),
            "x_cn": x_cn,
        }
        for p, w in (("e", pe), ("d", pd)):
            for key in ("wk", "wq", "wv", "pos", "bq", "gvb"):
                m[f"{key}_{p}"] = w[key]
        in_maps.append(m)

    res = run_bass_kernel_spmd(nc, in_maps, core_ids=list(range(B)))
    out = np.stack(
        [res.results[b]["out_cn"].reshape(C, H, W) for b in range(B)], axis=0
    )
    return out.astype(np.float32)


if __name__ == "__main__":
    import reference

    ins = {k: np.asarray(v) for k, v in reference.setup_inputs().items()}
    got = kernel(**ins)
    exp = np.asarray(reference.reference(**ins))
    err = np.abs(got - exp).max() / (np.abs(exp).max() + 1e-30)
    print("abs-rel err:", err)
